# revision 26
# baseline (speedup 1.0000x reference)
"""Multi-head self-attention with positional bias, sharded over 8 NeuronCores.

Sharding: head-parallel. Core h computes head h for all batches; the full
output is the sum of the 8 per-core partials (row-parallel Wout), reduced on
host.

Device kernel (per core), fp16 matmul inputs / fp32 PSUM accumulation:
  - scores are computed TRANSPOSED: ST[j, i] = k_j . q_i so exp's output is
    directly the layout the attention*V matmul needs.
  - the positional bias never touches the PE: host ships E = exp(bias^T) and
    the device computes P~ = exp(ST) * E with a 2x-mode fp16 DVE multiply.
  - FLIPPED PV: out[i, d] = sum_j P~[j, i] V[j, d] is computed with the P~
    block as the STATIONARY operand (lhsT = prod[:, i-chunk]) and V as the
    moving operand (free size 64). Ldweights is free-size-free, so this runs
    the PV contraction at 64 cycles per (i-chunk, j-tile) instead of the 512
    a V-stationary formulation pays -- full PE utilization.
  - softmax denominator: a second 1-column matmul per chunk with a ones
    vector as rhs (same stationary weights) accumulates sum_j P~[j, i] into
    a shared [128, 16] PSUM strip. Its reciprocal is applied per-partition
    during the *output* evacuation (tokens are on partitions there), fusing
    normalize into the copy.
  - PV result is [token, d]; the out-projection needs [d, token] stationary
    slices, so each sweep's PV block is transposed SBUF->SBUF on the DMA
    xbar (14ns/tile, off all compute engines). The last sweep uses a PE
    transpose instead to cut the exposed tail latency.
  - engine budget: Act = exp only; DVE = prod multiply + evacs + recip;
    Pool (GpSimd) = normalize+evacuate of the output projection; PE warms up
    with dummy matmuls so the p-state ramp completes before real work.
  - PSUM (8 banks): scores 2x[128,1024]f32 (4) + PV accum 2x[128,512] (2) +
    den/tail-transpose strip [128,512] (1) + out-proj [128,512] (1).
"""

import numpy as np
from contextlib import ExitStack

import concourse.bass as bass
import concourse.bacc as bacc
import concourse.mybir as mybir
import concourse.tile as tile
from concourse.bass_utils import run_bass_kernel_spmd

HEADS = 8
DH = 64
B, N, D = 4, 2048, 512
SCALE = DH ** -0.5
N_CORES = 8

F32 = mybir.dt.float32
F16 = mybir.dt.float16
MUL = mybir.AluOpType.mult


def build_nc(b=B, n=N, d=D, n_cores=1):
    """Per-core Bass program (SPMD: per-head differences come in via inputs)."""
    assert b % 2 == 0 and n % 512 == 0 and d % 128 == 0
    T = b * n
    CC = d // 128        # contraction chunks for projections
    NJ = n // 128        # key tiles (j)
    IC = 512
    NIC = n // IC        # i-chunks of 512
    NIP = NIC // 2       # i-groups of 1024
    NPAIR = b // 2
    NCH = 1024 // 128    # token chunks per step's i-window (8)

    nc = bacc.Bacc("TRN2", target_bir_lowering=False, debug=False,
                   num_devices=n_cores)
    qT = nc.declare_dram_parameter("qT", [d, T], F16, isOutput=False)
    eb = nc.declare_dram_parameter("eb", [n, n], F16, isOutput=False)
    wqk = nc.declare_dram_parameter("wqk", [128, CC * 2 * DH], F16, isOutput=False)
    wv = nc.declare_dram_parameter("wv", [128, CC * DH], F16, isOutput=False)
    wout = nc.declare_dram_parameter("wout", [DH, d], F16, isOutput=False)
    ident = nc.declare_dram_parameter("ident", [128, 128], F16, isOutput=False)
    out = nc.declare_dram_parameter("out", [T, d], F16, isOutput=True)

    with ExitStack() as ctx:
        tc = ctx.enter_context(tile.TileContext(nc))

        const = ctx.enter_context(tc.tile_pool(name="const", bufs=1))
        qk_pool = ctx.enter_context(tc.tile_pool(name="qkT", bufs=1))
        v_pool = ctx.enter_context(tc.tile_pool(name="v", bufs=1))
        e_pool = ctx.enter_context(tc.tile_pool(name="ebias", bufs=1))
        qt_pool = ctx.enter_context(tc.tile_pool(name="qt", bufs=8))
        p_pool = ctx.enter_context(tc.tile_pool(name="pexp", bufs=5))
        pr_pool = ctx.enter_context(tc.tile_pool(name="prod", bufs=10))
        pv_pool = ctx.enter_context(tc.tile_pool(name="pvsb", bufs=3))
        pvt_pool = ctx.enter_context(tc.tile_pool(name="pvt", bufs=3))
        rr_pool = ctx.enter_context(tc.tile_pool(name="rr", bufs=4))
        osb_pool = ctx.enter_context(tc.tile_pool(name="osb", bufs=6))
        # PSUM (8 banks): stA 2 + stB 2 + ots 2x1 + misc 1 + scratch 1.
        # Score tiles are two PINNED banks (tags stA/stB) so their WAR chain
        # is a clean double-buffer: score(s) waits exp(s-2) only. A shared
        # rotating pool would let proj/po allocations shift the rotation and
        # couple scores to exp(s-1).
        st_pool = ctx.enter_context(tc.tile_pool(name="st", bufs=1, space="PSUM"))
        ots_pool = ctx.enter_context(tc.tile_pool(name="ots", bufs=1, space="PSUM"))
        misc_pool = ctx.enter_context(tc.tile_pool(name="misc", bufs=1, space="PSUM"))
        po_pool = ctx.enter_context(tc.tile_pool(name="scr", bufs=1, space="PSUM"))

        zbias = const.tile([128, 1], F32, tag="zbias")
        nc.vector.memset(zbias, 0.0)
        ones1 = const.tile([128, 1], F16, tag="ones1")
        nc.vector.memset(ones1, 1.0)
        wsrc = const.tile([128, 64], F16, tag="wsrc")
        nc.vector.memset(wsrc, 0.0)

        wqk_sb = const.tile([128, CC, 2 * DH], F16, tag="wqk")
        nc.sync.dma_start(out=wqk_sb, in_=wqk[:, :].rearrange("p (c e) -> p c e", c=CC))
        wv_sb = const.tile([128, CC, DH], F16, tag="wv")
        # wout duplicated on partitions 64-127: the transposed PV tile holds
        # odd token-chunks there, and matmul operands must share a base
        # partition.
        wout_sb = const.tile([128, d], F16, tag="wout")
        ident_sb = const.tile([128, 128], F16, tag="ident")

        qT_sb = [qk_pool.tile([DH, n], F16, tag=f"qT{bb}", name=f"qT{bb}") for bb in range(b)]
        kT_sb = [qk_pool.tile([DH, n], F16, tag=f"kT{bb}", name=f"kT{bb}") for bb in range(b)]
        v_sb = [v_pool.tile([128, NJ * DH], F16, tag=f"v{bb}", name=f"v{bb}") for bb in range(b)]

        # den strip + tail-transpose scratch share one PSUM bank:
        # cols 0-15 = per-sweep-parity softmax denominators; cols 128-383
        # (bitcast f16 -> 512 cols) = the tail's PE-transpose target.
        misc = misc_pool.tile([128, 512], F32, tag="misc")
        mf16 = misc[:, 128:384].bitcast(F16)

        # warmup: keep the PE busy from t~0 so the p-state ramp (3us to full
        # clock) completes during the DMA lead-in instead of during real work.
        def emit_warmup(k):
            po_w = po_pool.tile([128, IC], F32, tag="scr", name="warm")
            for _ in range(k):
                nc.tensor.matmul(po_w[0:64, 0:64], lhsT=wsrc[:, 0:64],
                                 rhs=wsrc[:, 0:64], start=True, stop=True,
                                 skip_group_check=True)

        # ---------------- DMA staging ----------------
        e_sb = []

        def load_e_tiles(j0, j1, half=None):
            h = min(1024, n)
            for jt in range(j0, j1):
                if half in (None, 0):
                    t = e_pool.tile([128, n], F16, tag=f"eb{jt}", name=f"eb{jt}")
                    e_sb.append(t)
                t = e_sb[jt]
                if half is None:
                    nc.sync.dma_start(out=t, in_=eb[jt * 128:(jt + 1) * 128, :])
                elif half == 0:
                    nc.sync.dma_start(out=t[:, 0:h],
                                      in_=eb[jt * 128:(jt + 1) * 128, 0:h])
                elif h < n:
                    nc.sync.dma_start(out=t[:, h:n],
                                      in_=eb[jt * 128:(jt + 1) * 128, h:n])

        def load_qt(bb, split=1):
            qt_c = [qt_pool.tile([128, n], F16, tag="qt", name=f"qt{bb}_{c}")
                    for c in range(CC)]
            w = n // split
            for p in range(split):
                for c in range(CC):
                    nc.sync.dma_start(
                        out=qt_c[c][:, p * w:(p + 1) * w],
                        in_=qT[c * 128:(c + 1) * 128,
                               bb * n + p * w:bb * n + (p + 1) * w])
            return qt_c

        def emit_proj_qk(bb, qt_c, qh, act_evac=False, bank="scr"):
            pool_, tg = (po_pool, "scr") if bank == "scr" else (ots_pool, bank)
            ps = pool_.tile([128, IC], F32, tag=tg, name=f"pqk{bb}_{qh}")
            acols = slice(qh * IC, (qh + 1) * IC)
            for c in range(CC):
                nc.tensor.matmul(ps[:, 0:IC], lhsT=wqk_sb[:, c, :],
                                 rhs=qt_c[c][:, acols],
                                 start=(c == 0), stop=(c == CC - 1),
                                 skip_group_check=True)

            def evac():
                if act_evac:  # Act engine is idle in the lead-in
                    nc.scalar.copy(qT_sb[bb][:, acols], ps[0:DH, 0:IC])
                    nc.vector.tensor_copy(kT_sb[bb][:, acols], ps[DH:128, 0:IC])
                else:
                    nc.vector.tensor_copy(qT_sb[bb][:, acols], ps[0:DH, 0:IC])
                    nc.vector.tensor_copy(kT_sb[bb][:, acols], ps[DH:128, 0:IC])
            return evac

        def emit_proj_v(bb, qt_c, vh, act_evac=False, bank="scr"):
            # v: 8 token tiles side by side in one [128, 512] psum chunk
            HT = NJ // 2
            pool_, tg = (po_pool, "scr") if bank == "scr" else (ots_pool, bank)
            psv = pool_.tile([128, IC], F32, tag=tg, name=f"pv{bb}_{vh}")
            for t8 in range(HT):
                tt = vh * HT + t8
                for c in range(CC):
                    nc.tensor.matmul(psv[:, t8 * DH:(t8 + 1) * DH],
                                     lhsT=qt_c[c][:, tt * 128:(tt + 1) * 128],
                                     rhs=wv_sb[:, c, :],
                                     start=(c == 0), stop=(c == CC - 1),
                                     skip_group_check=True)
            vdst = v_sb[bb].rearrange("p (t w) -> p t w", w=DH)[:, vh * HT:(vh + 1) * HT, :]
            src = psv[:, 0:HT * DH].rearrange("p (t e) -> p t e", e=DH)

            def evac():
                if act_evac:
                    nc.scalar.copy(vdst, src)
                else:
                    nc.vector.tensor_copy(vdst, src)
            return evac

        # ---- lead-in DMA order: first qh piece of batch 0 ASAP, then E ----
        emit_warmup(8)
        qt_b = {0: load_qt(0, split=2)}  # col pieces, c-major: qh0+qh1 land first
        emit_warmup(30)
        # batch-0 qh0/qh1 projections ASAP (lead-in); qh2+ are woven.
        emit_proj_qk(0, qt_b[0], 0, act_evac=True)()
        nc.sync.dma_start(out=wv_sb, in_=wv[:, :].rearrange("p (c e) -> p c e", c=CC))
        load_e_tiles(0, min(4, NJ), half=0)
        emit_proj_qk(0, qt_b[0], 1, act_evac=True, bank="ot1")()
        nc.sync.dma_start(out=wout_sb[0:DH, :], in_=wout[:, :])
        nc.sync.dma_start(out=wout_sb[DH:128, :], in_=wout[:, :])
        nc.sync.dma_start(out=ident_sb, in_=ident[:, :])
        qt_b[1] = load_qt(1, split=2)
        load_e_tiles(min(4, NJ), NJ, half=0)
        if b > 2:
            qt_b[2] = load_qt(2, split=1)
        load_e_tiles(0, NJ, half=1)
        if b > 2:
            qt_b[3] = load_qt(3, split=1)

        # ---------------- main stream ----------------
        # Every prod multiply is split by columns: DVE cols 0-639 (2x mode),
        # Pool cols 640-1023. GpSimd cannot touch PSUM so the po evacuation
        # must stay on DVE; the column split keeps DVE under the Act exp
        # ceiling with a uniform, low-latency Pool load (a per-step whole-prod
        # Pool offload made Pool the pipeline pacer).
        exp_fn = mybir.ActivationFunctionType.Exp
        DEPTH = 5
        PSPL = 640         # prod column split: DVE 640 / Pool 384 cols

        steps = [(ip, pair, jt, lb)
                 for pair in range(NPAIR) for ip in range(NIP)
                 for lb in range(2) for jt in range(NJ)]
        n_steps = len(steps)
        n_sweeps = n_steps // NJ

        pv_q = []          # (release_step, fn)
        extra_q = []       # (release_step, fn)
        last_pv_rel = [0]  # global monotonic PV release (keeps psum order:
                           # a sweep's last PVs must precede the next sweep's
                           # bank-resetting start in emission order)

        ots_sw = {}        # sweep -> psum tile [128, 512]
        pv_sw = {}         # sweep -> pv_sb tile (fp16, unnormalized)
        pvt_sw = {}        # sweep -> transposed [128, 512] (d-major halves)
        rr_sw = {}         # sweep -> [128, 8] f32 reciprocal denominators
        osb_sw = {}        # (sweep, cpair) -> osb tile

        def sweep_geo(sw):
            blk, lb = sw // 2, sw % 2
            pair, ip = blk // NIP, blk % NIP
            return 2 * pair + lb, ip

        def emit_pv(sw, bb, jt, prod):
            # one start/stop per PSUM bank per sweep: start marks the whole
            # 2KB zero-region pending (first write overwrites, later writes
            # accumulate), so a second start inside the bank would wipe
            # already-accumulated chunks.
            def fn():
                par8 = (sw % 2) * 8
                for c in range(NCH):
                    lt = prod[:, c * 128:(c + 1) * 128]
                    nc.tensor.matmul(
                        ots_sw[sw][:, c * DH:(c + 1) * DH],
                        lhsT=lt, rhs=v_sb[bb][:, jt * DH:(jt + 1) * DH],
                        start=(jt == 0 and c == 0),
                        stop=(jt == NJ - 1 and c == NCH - 1),
                        skip_group_check=True)
                    nc.tensor.matmul(
                        misc[:, par8 + c:par8 + c + 1],
                        lhsT=lt, rhs=ones1,
                        start=(jt == 0 and c == 0),
                        stop=(jt == NJ - 1 and c == NCH - 1),
                        skip_group_check=True)
            return fn

        def emit_evac_recip(sw):
            # recip BEFORE the pv evac: downstream transposes depend on the
            # evac, so anything that overwrites the den bank (a later sweep's
            # start, the tail PE transposes) is ordered after the recip read.
            def fn():
                par8 = (sw % 2) * 8
                rr = rr_pool.tile([128, 8], F32, tag="rr", name=f"rr{sw}")
                nc.vector.reciprocal(rr, misc[:, par8:par8 + 8])
                rr_sw[sw] = rr
                pv = pv_pool.tile([128, 512], F16, tag="pv", name=f"pv{sw}")
                nc.vector.tensor_copy(pv, ots_sw[sw])
                pv_sw[sw] = pv
            return fn

        def emit_transpose(sw):
            def fn():
                pvt = pvt_pool.tile([128, 512], F16, tag="pvt", name=f"pvt{sw}")
                for q in range(4):
                    nc.sync.dma_start_transpose(
                        pvt[:, q * 128:(q + 1) * 128],
                        pv_sw[sw][:, q * 128:(q + 1) * 128])
                pvt_sw[sw] = pvt
            return fn

        def emit_tail_transpose(sw):
            def fn():
                for q in range(4):
                    nc.tensor.transpose(
                        mf16[:, q * 128:(q + 1) * 128],
                        pv_sw[sw][:, q * 128:(q + 1) * 128],
                        ident_sb)
                pvt = pvt_pool.tile([128, 512], F16, tag="pvt", name=f"pvt{sw}")
                nc.vector.tensor_copy(pvt, mf16)
                pvt_sw[sw] = pvt
            return fn

        def emit_po(sw, c, tail=False):
            # tail: double-bank the po psum (scratch + the long-freed ot0
            # bank) and split the normalize-evacs DVE/Act so the final po
            # chain pipelines instead of serializing at ~1.1us/unit.
            def fn():
                bb, ip = sweep_geo(sw)
                pvt = pvt_sw[sw]
                lhsT = pvt[(c % 2) * DH:(c % 2 + 1) * DH,
                           (c // 2) * 128:(c // 2 + 1) * 128]
                if tail:
                    tg = ["scr", "ot0", "ot1", "st1"][c % 4]
                    pool_ = {"scr": po_pool, "ot0": ots_pool, "ot1": ots_pool,
                             "st1": st_pool}[tg]
                    po_t = pool_.tile([128, d], F32, tag=tg, name=f"po{sw}_{c}")
                else:
                    po_t = po_pool.tile([128, d], F32, tag="scr",
                                        name=f"po{sw}_{c}")
                wo = wout_sb[(c % 2) * DH:(c % 2 + 1) * DH, :]
                nc.tensor.matmul(po_t, lhsT=lhsT, rhs=wo,
                                 start=True, stop=True, skip_group_check=True)
                if c % 2 == 0:
                    osb_sw[(sw, c // 2)] = osb_pool.tile(
                        [128, 2 * d], F16, tag="osb", name=f"osb{sw}_{c//2}")
                osb = osb_sw[(sw, c // 2)]
                dst = osb[:, (c % 2) * d:(c % 2 + 1) * d]
                rr_c = rr_sw[sw][:, c:c + 1]
                with nc.allow_low_precision("fused softmax-normalize evac"):
                    if tail and c % 2 == 1:  # Act is idle at the tail
                        nc.scalar.activation(dst, po_t,
                                             mybir.ActivationFunctionType.Copy,
                                             scale=rr_c)
                    else:
                        nc.vector.tensor_scalar(dst, po_t, rr_c, None, MUL)
            return fn

        def emit_store(sw, cpair, half=None):
            def fn():
                bb, ip = sweep_geo(sw)
                r0 = bb * n + ip * 1024 + cpair * 256
                osb = osb_sw[(sw, cpair)]
                if half is None:
                    nc.sync.dma_start(
                        out=out[r0:r0 + 256, :].rearrange("(t p) d -> p t d", p=128),
                        in_=osb.rearrange("p (t d) -> p t d", t=2))
                else:
                    rh = r0 + half * 128
                    nc.sync.dma_start(
                        out=out[rh:rh + 128, :],
                        in_=osb[:, half * d:(half + 1) * d])
            return fn

        # projection weave: (batch, unit) at chosen steps. units: qh 0..NQH-1,
        # then v halves NQH, NQH+1.
        NQH = n // IC
        proj_w = {}

        def sched_weave(bb, positions):
            for u, s in enumerate(positions):
                proj_w[s] = (bb, u)

        def sched_weave_units(bb, units, positions):
            for u, s in zip(units, positions):
                proj_w[s] = (bb, u)

        if NJ == 16:  # full config: batch 0 lead-in did qh0/qh1
            sched_weave_units(0, [2, 3, 4, 5], [1, 3, 2, 4])   # qh2 qh3 v0 v1
            sched_weave(1, [5, 7, 9, 11, 17, 18])              # qh0..3 v0 v1
            if b > 2:
                sched_weave(2, [37, 38, 39, 40, 53, 54])
                sched_weave(3, [55, 56, 69, 70, 71, 72])
        else:  # small config: NQH == 2
            sched_weave_units(0, [2, 3], [1, 2])               # v0 v1
            sched_weave(1, [3, 4, 5, 6])                       # qh0 qh1 v0 v1

        for s in range(n_steps + 24):
            if s in proj_w:
                pb, part = proj_w[s]
                # dense early region: alternate scratch/ot1 so proj units
                # double-buffer instead of serializing on one bank (ot1 is
                # free until sweep 1's first PV claims it)
                bank = "ot1" if (s <= NJ - 2 and s % 2 == 1) else "scr"
                if part < NQH:
                    ev = emit_proj_qk(pb, qt_b[pb], part, bank=bank)
                else:
                    ev = emit_proj_v(pb, qt_b[pb], part - NQH, bank=bank)
                extra_q.append((s + 1, ev))
            if s < n_steps:
                ip, pair, jt, lb = steps[s]
                sw = s // NJ
                bb = 2 * pair + lb
                if jt == 0:
                    ots_sw[sw] = ots_pool.tile([128, 512], F32, tag=f"ot{sw % 2}",
                                               name=f"ots{sw}")
                st = st_pool.tile([128, 2 * IC], F32, tag=f"st{s % 2}", name="st")
                for il in range(2):
                    ic = ip * 2 + il
                    nc.tensor.matmul(
                        st[:, il * IC:(il + 1) * IC],
                        lhsT=kT_sb[bb][:, jt * 128:(jt + 1) * 128],
                        rhs=qT_sb[bb][:, ic * IC:(ic + 1) * IC],
                        start=True, stop=True, skip_group_check=True)
                pexp = p_pool.tile([128, 2 * IC], F16, tag="pexp")
                nc.scalar.activation(pexp, st, exp_fn, bias=zbias)
                prod = pr_pool.tile([128, 2 * IC], F16, tag="prod")
                off = ip * 1024
                if s >= n_steps - 2:  # wind-down: skip Pool's 0.9us latency
                    nc.vector.tensor_tensor(
                        prod, pexp, e_sb[jt][:, off:off + 1024], MUL)
                else:
                    nc.vector.tensor_tensor(
                        prod[:, 0:PSPL], pexp[:, 0:PSPL],
                        e_sb[jt][:, off:off + PSPL], MUL)
                    nc.gpsimd.tensor_tensor(
                        prod[:, PSPL:1024], pexp[:, PSPL:1024],
                        e_sb[jt][:, off + PSPL:off + 1024], MUL)
                rel = max(s + DEPTH, last_pv_rel[0])
                last_pv_rel[0] = rel
                pv_q.append((rel, emit_pv(sw, bb, jt, prod)))
                if jt == NJ - 1:
                    base = rel
                    tail = sw == n_sweeps - 1
                    if tail:
                        extra_q.append((base, emit_evac_recip(sw)))
                        extra_q.append((base, emit_tail_transpose(sw)))
                        for c in range(NCH):
                            extra_q.append((base + 1 + c // 2, emit_po(sw, c, tail=True)))
                            if c % 2 == 1:
                                extra_q.append((base + 1 + c // 2, emit_store(sw, c // 2)))
                    else:
                        extra_q.append((base, emit_evac_recip(sw)))
                        extra_q.append((base + 1, emit_transpose(sw)))
                        for c in range(NCH):
                            pc = base + 5 + c + (c + 1) // 2
                            extra_q.append((pc, emit_po(sw, c)))
                            if c % 2 == 1:
                                extra_q.append((pc + 1, emit_store(sw, c // 2)))
            for q in (pv_q, extra_q):
                ready = [f for r, f in q if r <= s]
                q[:] = [(r, f) for r, f in q if r > s]
                for f in ready:
                    f()
    nc.compile()
    return nc


def make_in_maps(query, pos_bias, Wq, Wk, Wv, Wout, n_cores=N_CORES):
    """Host-side sharding/layout prep. Head h -> core h."""
    query = np.asarray(query, dtype=np.float32)
    pos_bias = np.asarray(pos_bias, dtype=np.float32)
    Wq = np.asarray(Wq, dtype=np.float32)
    Wk = np.asarray(Wk, dtype=np.float32)
    Wv = np.asarray(Wv, dtype=np.float32)
    Wout = np.asarray(Wout, dtype=np.float32)

    b, n, d = query.shape
    qT = np.ascontiguousarray(query.reshape(b * n, d).T.astype(np.float16))
    wq_s = Wq * np.float32(SCALE)
    ident = np.eye(128, dtype=np.float16)
    in_maps = []
    for h in range(n_cores):
        sl = slice(h * DH, (h + 1) * DH)
        wqk_h = np.concatenate([wq_s[:, sl], Wk[:, sl]], axis=1).astype(np.float16)
        wv_h = Wv[:, sl].astype(np.float16)
        # device layout: [partition, c, e] (c = 128-row contraction chunk)
        cc = d // 128
        wqk_h = np.ascontiguousarray(
            wqk_h.reshape(cc, 128, -1).transpose(1, 0, 2).reshape(128, -1))
        wv_h = np.ascontiguousarray(
            wv_h.reshape(cc, 128, -1).transpose(1, 0, 2).reshape(128, -1))
        in_maps.append({
            "qT": qT,
            "eb": np.ascontiguousarray(np.exp(pos_bias[h].T).astype(np.float16)),
            "wqk": wqk_h,
            "wv": wv_h,
            "wout": np.ascontiguousarray(Wout[sl, :].astype(np.float16)),
            "ident": ident,
        })
    return in_maps


def run_device(in_maps, b=B, n=N, d=D, trace=False, **kw):
    nc = build_nc(b, n, d, n_cores=len(in_maps))
    return run_bass_kernel_spmd(nc, in_maps, list(range(len(in_maps))), trace=trace, **kw)


def assemble(results, b=B, n=N, d=D):
    acc = np.zeros((b * n, d), dtype=np.float32)
    for r in results:
        acc += r["out"]
    return acc.reshape(b, n, d)


def kernel(query, pos_bias, Wq, Wk, Wv, Wout):
    in_maps = make_in_maps(query, pos_bias, Wq, Wk, Wv, Wout)
    res = run_device(in_maps)
    return assemble(res.results)
